# revision 52
# baseline (speedup 1.0000x reference)
"""AutoCorrelation (Autoformer) Trainium2 kernel.

Per (batch, head, depth-row) r of the projected series qt, kt (length L=2048):
R = irfft(rfft(qt) * conj(rfft(kt))); top-15 lags of R; softmax over those
R-values; out = (sum_i w_i * roll(qt, -idx_i)) @ Wo + bo.

Strategy (one batch per core, 8 cores):
  - rfft via two matmul stages on the RAW q/k (FFT commutes with the feature
    projection), so the Wq/Wk projection happens in the frequency domain and
    no input transposes are needed.
  - aggregation in frequency domain: sum_i w_i*roll(qt,-m_i) = irfft(Qf * Sf),
    S a sparse spike train built with gpsimd local_scatter.
  - top-k via DVE max/max_index/match_replace; softmax via ACT exp.
  - inverse rfft: packed 128-deep stage-1 matmuls ([re|im] joint operand),
    SBUF-resident corner turn, permuted time order (unscrambled only in the
    final output DMA / top-k index arithmetic).
"""

import functools
import math
import os
import sys

import numpy as np

sys.path.insert(0, "/opt/trn_rl_repo")

import concourse.bacc as bacc  # noqa: E402
import concourse.mybir as mybir  # noqa: E402
from concourse.bass_utils import run_bass_kernel_spmd  # noqa: E402
from concourse.tile import TileContext  # noqa: E402

B, L, D, H = 8, 2048, 512, 8
TOPK = int(2 * math.log(L))  # 15
NW = 16
F2 = 65   # rfft second-factor range: f = f1 + 16*f2, f1<16, f2<65
F2P = 66  # padded (col 65 zero) so [re|im] blocks are 4B-aligned
NC2 = 2 * F2P  # 132 cols per f1 in a spectrum tile
NCORES = 8

f32 = mybir.dt.float32
f32r = mybir.dt.float32r
f16 = mybir.dt.float16
i16 = mybir.dt.int16
i32 = mybir.dt.int32
u32 = mybir.dt.uint32
AF = mybir.ActivationFunctionType
ALU = mybir.AluOpType
AX = mybir.AxisListType


@functools.lru_cache(maxsize=1)
def _consts():
    c = {}
    # q/k variant: partition p = t1*8 + t2l (t1-major, matches XA dma dim order)
    Mre = np.zeros((NW, 128, 128), np.float64)
    Mim = np.zeros((NW, 128, 128), np.float64)
    # S variant: partition p = t2l*16 + t1 (t2l-major, matches scratch relayout)
    MreS = np.zeros((NW, 128, 128), np.float64)
    MimS = np.zeros((NW, 128, 128), np.float64)
    for w in range(NW):
        for t2l in range(8):
            for t1 in range(16):
                t = t1 * 128 + 8 * w + t2l
                f1v = np.arange(16)
                ang = 2 * np.pi * f1v * t / L
                m = t2l * 16 + f1v
                Mre[w, t1 * 8 + t2l, m] = np.cos(ang)
                Mim[w, t1 * 8 + t2l, m] = -np.sin(ang)
                MreS[w, t2l * 16 + t1, m] = np.cos(ang)
                MimS[w, t2l * 16 + t1, m] = -np.sin(ang)
    c["mreq16"] = Mre.reshape(NW * 128, 128).astype(np.float16)
    c["mimq16"] = Mim.reshape(NW * 128, 128).astype(np.float16)
    c["mre16"] = MreS.reshape(NW * 128, 128).astype(np.float16)
    c["mim16"] = MimS.reshape(NW * 128, 128).astype(np.float16)
    # forward second stage, padded to F2P (col 65 of each quadrant = 0)
    t2 = np.arange(128)[:, None]
    f2 = np.arange(F2P)[None, :]
    Vre = np.cos(2 * np.pi * t2 * f2 / 128)
    Vim = -np.sin(2 * np.pi * t2 * f2 / 128)
    Vre[:, F2:] = 0.0
    Vim[:, F2:] = 0.0
    c["vcat"] = np.concatenate([Vre, Vim, -Vim, Vre], axis=1).astype(np.float16)
    # inverse stage-1 packed stationaries: J rows = [re f2 0..63 | im f2 0..63]
    f2c = np.arange(64)[:, None]
    t2p = np.arange(128)[None, :]
    Ure = (2.0 / L) * np.cos(2 * np.pi * f2c * t2p / 128)
    Uim = (2.0 / L) * np.sin(2 * np.pi * f2c * t2p / 128)
    c["ua"] = np.concatenate([Ure, -Uim], axis=0).astype(np.float16)  # -> C_re
    c["ub"] = np.concatenate([Uim, Ure], axis=0).astype(np.float16)   # -> C_im
    # f1=0 variants: J0 rows = [re f2 0..64 (64=Nyquist) | im f2 1..63].
    # Nyquist row (f=1024): (2/L)*cos(pi*t2) with the 0.5 halving folded in.
    u64row = (1.0 / L) * np.cos(np.pi * t2p)
    c["ua0"] = np.concatenate([Ure, u64row, -Uim[1:64]], axis=0).astype(np.float16)
    c["ub0"] = np.concatenate([Uim, np.zeros_like(u64row), Ure[1:64]], axis=0).astype(
        np.float16
    )
    W2 = np.zeros((NW, 128, 256), np.float64)
    for wp in range(NW):
        for t2l in range(8):
            for f1 in range(16):
                p = t2l * 16 + f1
                t1pp = np.arange(16)
                t = 128 * t1pp + 8 * wp + t2l
                ang = 2 * np.pi * f1 * t / L
                W2[wp, p, t2l * 16 + t1pp] = np.cos(ang)
                W2[wp, p, 128 + t2l * 16 + t1pp] = -np.sin(ang)
    c["w216"] = W2.reshape(NW * 128, 256).astype(np.float16)
    # packed small consts: [ident | ua | ub | ua0 | ub0] as one [128, 640] f16
    c["smallc"] = np.concatenate(
        [np.eye(128, dtype=np.float16), c["ua"], c["ub"], c["ua0"], c["ub0"]],
        axis=1,
    )
    # packed S-fft stage-A matrices: [mre16 cols | mim16 cols] -> [128, 4096]
    return c


def _build(dbg=()):
    nc = bacc.Bacc("TRN2", target_bir_lowering=False, num_devices=NCORES)

    par = {}

    def dram(name, shape, dt, out=False):
        par[name] = nc.declare_dram_parameter(name, list(shape), dt, isOutput=out)
        return par[name]

    q_in = dram("q", [L, D], f32r)
    k_in = dram("k", [L, D], f32r)
    mre_in = dram("mreq16", [NW * 128, 128], f16)
    mim_in = dram("mimq16", [NW * 128, 128], f16)
    wq_in = dram("wq16", [D, D], f16)
    wk_in = dram("wk16", [D, D], f16)
    wo_in = dram("wo16", [D, D], f16)
    bq_in = dram("bq2", [4, 128], f32)
    bk_in = dram("bk2", [4, 128], f32)
    bo_in = dram("borep", [128, D], f32)
    mre16_in = dram("mre16", [NW * 128, 128], f16)
    mim16_in = dram("mim16", [NW * 128, 128], f16)
    v_in = dram("vcat", [128, 4 * F2P], f16)
    smallc_in = dram("smallc", [128, 640], f16)
    w2_in = dram("w216", [NW * 128, 256], f16)
    out_ext = dram("out", [L, D], f32, out=True)

    with TileContext(nc) as tc:
        with tc.tile_pool(name="consts", bufs=1) as consts:
            vcat = consts.tile([128, 4 * F2P], f16, name="vcat")
            wq16 = consts.tile([128, 4 * D], f16, name="wq16")
            wk16 = consts.tile([128, 4 * D], f16, name="wk16")
            bq_sb = consts.tile([128, 4], f32, name="bq_sb")
            bk_sb = consts.tile([128, 4], f32, name="bk_sb")
            smallc = consts.tile([128, 640], f16, name="smallc")
            ident = smallc[:, 0:128]
            ua_t = smallc[:, 128:256]
            ub_t = smallc[:, 256:384]
            ua0_t = smallc[:, 384:512]
            ub0_t = smallc[:, 512:640]
            m_re16 = consts.tile([128, NW * 128], f16, name="m_re16")
            m_im16 = consts.tile([128, NW * 128], f16, name="m_im16")
            w2c = consts.tile([128, NW * 256], f16, name="w2c")
            woc = consts.tile([128, 4 * D], f16, name="woc")
            bo_sb = consts.tile([128, D], f32, name="bo_sb")

            def load_consts():
                # early (scalar queue): only what the q/k fft + proj need soon
                nc.scalar.dma_start(out=vcat[:], in_=v_in[:])
                nc.scalar.dma_start(out=smallc[:], in_=smallc_in[:])
                nc.scalar.dma_start(
                    out=wq16[:], in_=wq_in[:].rearrange("(fc p) c -> p fc c", fc=4)
                )
                nc.scalar.dma_start(
                    out=wk16[:], in_=wk_in[:].rearrange("(fc p) c -> p fc c", fc=4)
                )
                nc.scalar.dma_start(out=bq_sb[:], in_=bq_in[:].rearrange("c p -> p c"))
                nc.scalar.dma_start(out=bk_sb[:], in_=bk_in[:].rearrange("c p -> p c"))

            def load_consts_late():
                # bulk consts for the back half, issued once the q fft is
                # underway (scalar queue is free again after the A(q) evacs)
                nc.scalar.dma_start(
                    out=w2c[:], in_=w2_in[:].rearrange("(w p) c -> p w c", w=NW)
                )
                nc.scalar.dma_start(
                    out=m_re16[:], in_=mre16_in[:].rearrange("(w p) c -> p w c", w=NW)
                )
                nc.scalar.dma_start(
                    out=m_im16[:], in_=mim16_in[:].rearrange("(w p) c -> p w c", w=NW)
                )
                nc.scalar.dma_start(
                    out=woc[:], in_=wo_in[:].rearrange("(fc p) c -> p fc c", fc=4)
                )
                nc.scalar.dma_start(out=bo_sb[:], in_=bo_in[:])

            _pipeline(nc, tc, locals())

    nc.compile()
    return nc, par


def _pipeline(nc, tc, E):
    q_in, k_in = E["q_in"], E["k_in"]
    mre_in, mim_in = E["mre_in"], E["mim_in"]
    out_ext = E["out_ext"]
    m_re16, m_im16 = E["m_re16"], E["m_im16"]
    vcat, w2c = E["vcat"], E["w2c"]
    ua_t, ub_t = E["ua_t"], E["ub_t"]
    ua0_t, ub0_t = E["ua0_t"], E["ub0_t"]
    wq16, wk16, woc = E["wq16"], E["wk16"], E["woc"]
    bq_sb, bk_sb, bo_sb = E["bq_sb"], E["bk_sb"], E["bo_sb"]
    load_consts = E["load_consts"]
    load_consts_late = E["load_consts_late"]
    ident = E["ident"]

    # ---------------- forward rfft (shared by q, k, S) ---------------------
    # Emitted as per-step closures so independent work can be interleaved in
    # program order (engines execute their queues strictly in order).
    def stageA_steps(name, get_xview, mr, mi, work_pool, psum_pool, bbuf):
        bview = bbuf[:].rearrange("p (ri f1 d) -> p ri f1 d", ri=2, f1=16)

        pending = []

        def flush(lag):
            # Emit relayout DMAs a couple of iterations late so their evac
            # dependencies are long satisfied by the time they reach the
            # queue head — a waiting DMA stalls everything behind it on its
            # in-order queue.
            while len(pending) > lag:
                w, ev = pending.pop(0)
                nc.scalar.dma_start(out=bview[8 * w : 8 * w + 8, 0], in_=ev[:, 0:512])
                nc.gpsimd.dma_start(out=bview[8 * w : 8 * w + 8, 1], in_=ev[:, 512:1024])

        def step(w):
            xv = get_xview(w)
            ev = work_pool.tile(
                [128, 1024], f16, name=f"ev{w % 5}", tag=f"ev{w % 5}"
            )
            par_ = psum_pool.tile([128, 512], f32, name=f"par{w % 3}", tag=f"par{w % 3}")
            pai = psum_pool.tile([128, 512], f32, name=f"pai{w % 3}", tag=f"pai{w % 3}")
            wsl = slice(w * 128, (w + 1) * 128)
            nc.tensor.matmul(par_[:], mr[:, wsl], xv, start=True, stop=True)
            nc.tensor.matmul(pai[:], mi[:, wsl], xv, start=True, stop=True)
            nc.scalar.copy(ev[:, 0:512], par_[:])
            nc.vector.tensor_copy(ev[:, 512:1024], pai[:])
            pending.append((w, ev))
            flush(2)
            if w == NW - 1:
                flush(0)

        return [lambda w=w: step(w) for w in range(NW)]

    def stageB_steps(name, sp, bbuf, psum_pool):
        def step(rc, f1):
            pb = psum_pool.tile([128, NC2], f32, name=f"pb{f1 % 2}", tag=f"pb{f1 % 2}")
            bsl = slice(f1 * D + rc * 128, f1 * D + rc * 128 + 128)
            isl = slice(NW * 512 + f1 * D + rc * 128, NW * 512 + f1 * D + rc * 128 + 128)
            nc.tensor.matmul(pb[:], bbuf[:, bsl], vcat[:, :NC2], start=True, stop=False)
            nc.tensor.matmul(pb[:], bbuf[:, isl], vcat[:, NC2:], start=False, stop=True)
            if (rc * 16 + f1) % 2 == 0:
                nc.scalar.copy(sp[rc][:, f1 * NC2 : (f1 + 1) * NC2], pb[:])
            else:
                nc.vector.tensor_copy(sp[rc][:, f1 * NC2 : (f1 + 1) * NC2], pb[:])

        return [lambda rc=rc, f1=f1: step(rc, f1) for rc in range(4) for f1 in range(16)]

    # ---------------- long-lived pools (opened in LIFO close order) --------
    _big_cm = tc.tile_pool(name="bigpool", bufs=1)
    big_pool = _big_cm.__enter__()
    _phq_cm = tc.tile_pool(name="projq", bufs=1)
    phq_pool = _phq_cm.__enter__()
    _spS_cm = tc.tile_pool(name="spSp", bufs=1)
    spS_pool = _spS_cm.__enter__()
    _svrb_cm = tc.tile_pool(name="svrb", bufs=1)
    svrb_pool = _svrb_cm.__enter__()
    _pk_cm = tc.tile_pool(name="projk", bufs=1)
    pk_pool = _pk_cm.__enter__()
    _raw_cm = tc.tile_pool(name="rawsp", bufs=1)
    rawsp_pool = _raw_cm.__enter__()

    # ---------------- q/k loads + fft + projection, interleaved ------------
    ncols = 16 * NC2  # 2112
    spect = {}
    proj = {}
    with tc.tile_pool(name="bigc", bufs=1) as bigc, \
         tc.tile_pool(name="xa", bufs=1) as xa_pool, \
         tc.tile_pool(name="fftwork", bufs=1) as work_pool, \
         tc.tile_pool(name="projtmp", bufs=1) as pj_tmp, \
         tc.tile_pool(name="fftpsum", bufs=1, space="PSUM") as psum_pool:
        m_re = bigc.tile([128, NW * 128], f16, name="m_re")
        m_im = bigc.tile([128, NW * 128], f16, name="m_im")
        nc.scalar.dma_start(
            out=m_re[:], in_=mre_in[:].rearrange("(w p) c -> p w c", w=NW)
        )
        nc.scalar.dma_start(
            out=m_im[:], in_=mim_in[:].rearrange("(w p) c -> p w c", w=NW)
        )
        load_consts()

        def make_get_xview(nm, src):
            srcv = src[:].rearrange(
                "(t1 wl t2l) d -> wl t1 t2l d", t1=16, wl=16, t2l=8
            )
            tiles = {}
            casts = {}

            def load(w):
                if w in tiles or w >= NW:
                    return
                xa = xa_pool.tile(
                    [128, D], f32r, name=f"xa{nm}{w}", tag="xa", bufs=4
                )
                nc.sync.dma_start(out=xa[:], in_=srcv[w])
                tiles[w] = xa

            def cast(w):
                # f32 -> f16 on the (otherwise idle) gpsimd compute path so
                # the stage-A matmuls run at full f16 PE rate
                if w in casts or w >= NW:
                    return
                xc = xa_pool.tile(
                    [128, D], f16, name=f"xc{nm}{w}", tag="xc", bufs=4
                )
                nc.gpsimd.tensor_copy(xc[:], tiles[w][:])
                casts[w] = xc

            def get(w):
                for ww in (w, w + 1, w + 2, w + 3):
                    load(ww)
                for ww in (w, w + 1, w + 2):
                    cast(ww)
                return casts[w][:]

            return get

        def proj_units(nm, wmat, bias):
            dstp = phq_pool if nm == "q" else pk_pool
            ph = [dstp.tile([128, ncols], f16, name=f"ph_{nm}_{ht}") for ht in range(4)]
            proj[nm] = ph

            def unit(ht, c0):
                cw = min(512, ncols - c0)
                pp = psum_pool.tile(
                    [128, 512], f32, name=f"pp{ht % 2}", tag=f"pb{ht % 2}"
                )
                for fc in range(4):
                    nc.tensor.matmul(
                        pp[:, :cw],
                        wmat[:, fc * D + ht * 128 : fc * D + ht * 128 + 128],
                        spect[nm][fc][:, c0 : c0 + cw],
                        start=(fc == 0),
                        stop=(fc == 3),
                    )
                if c0 == 0:
                    # bias: rfft(b*ones)[f=0] = L*b -> col 0 (f1=0, re, f2=0)
                    bb = pj_tmp.tile([128, 1], f32, name=f"bb_{nm}_{ht}", tag="bbias")
                    nc.vector.tensor_scalar_mul(bb[:], bias[:, ht : ht + 1], float(L))
                    nc.vector.tensor_tensor(
                        out=pp[:, 0:1], in0=pp[:, 0:1], in1=bb[:], op=ALU.add
                    )
                nc.scalar.copy(ph[ht][:, c0 : c0 + cw], pp[:, :cw])

            return [
                lambda ht=ht, c0=c0: unit(ht, c0)
                for ht in range(4)
                for c0 in range(0, ncols, 512)
            ]

        for nm in ("q", "k"):
            spect[nm] = [
                rawsp_pool.tile([128, 16 * NC2], f16, name=f"sp_{nm}_{rc}")
                for rc in range(4)
            ]
        bbuf_q = big_pool.tile([128, 2 * NW * 512], f16, name="bbuf_q", tag="big")
        bbuf_k = big_pool.tile([128, 2 * NW * 512], f16, name="bbuf_k", tag="big")
        sA_q = stageA_steps("q", make_get_xview("q", q_in), m_re, m_im,
                            work_pool, psum_pool, bbuf_q)
        sA_k = stageA_steps("k", make_get_xview("k", k_in), m_re, m_im,
                            work_pool, psum_pool, bbuf_k)
        sB_q = stageB_steps("q", spect["q"], bbuf_q, psum_pool)
        sB_k = stageB_steps("k", spect["k"], bbuf_k, psum_pool)
        pU_q = proj_units("q", wq16, bq_sb)
        pU_k = proj_units("k", wk16, bk_sb)

        for s in sA_q:
            s()
        load_consts_late()
        for s in sB_q:
            s()
        # proj(q) interleaved with A(k): the A(k) chain trickles through the
        # evac/DMA engines while proj keeps the PE busy.
        ak = 0
        for u in pU_q:
            if ak < NW:
                sA_k[ak]()
                ak += 1
            u()
        while ak < NW:
            sA_k[ak]()
            ak += 1
        for s in sB_k:
            s()
        for u in pU_k:
            u()
    _raw_cm.__exit__(None, None, None)

    # ---------------- fused complex product --------------------------------
    def specmul(zcat, a, b, conj_b, mul_pool):
        """a,b: [128, 16*132] f16 spectra. zcat: [128, 16*128] f16, per f1:
        cols [0:64] = Z_re f2 0..63, cols [64:128] = Z_im f2 0..63.
        For f1=0, col 64 is overwritten with Z_re f2=64 (Nyquist); the true
        im f2=0 there is exactly zero. DC (col 0) halved."""
        av = a.rearrange("p (f1 c) -> p f1 c", f1=16)
        bv = b.rearrange("p (f1 c) -> p f1 c", f1=16)
        ar, ai = av[:, :, 0:64], av[:, :, F2P : F2P + 64]
        br, bi = bv[:, :, 0:64], bv[:, :, F2P : F2P + 64]
        zv = zcat.rearrange("p (f1 c) -> p f1 c", f1=16)
        dre, dim = zv[:, :, 0:64], zv[:, :, 64:128]
        t1 = mul_pool.tile([128, 16 * 64], f16, name="smt1", tag="smt1")
        t2 = mul_pool.tile([128, 16 * 64], f16, name="smt2", tag="smt2")
        t1v = t1[:].rearrange("p (f1 c) -> p f1 c", f1=16)
        t2v = t2[:].rearrange("p (f1 c) -> p f1 c", f1=16)
        nc.vector.tensor_mul(t1v, ar, br)
        nc.vector.tensor_mul(t2v, ai, bi)
        if conj_b:
            nc.vector.tensor_add(dre, t1v, t2v)
        else:
            nc.vector.tensor_sub(dre, t1v, t2v)
        nc.vector.tensor_mul(t1v, ai, br)
        nc.vector.tensor_mul(t2v, ar, bi)
        if conj_b:
            nc.vector.tensor_sub(dim, t1v, t2v)
        else:
            nc.vector.tensor_add(dim, t1v, t2v)
        # Nyquist (f1=0, f2=64, re) -> col 64 of the f1=0 block
        n1 = mul_pool.tile([128, 1], f16, name="smn1", tag="smn1")
        n2 = mul_pool.tile([128, 1], f16, name="smn2", tag="smn2")
        nc.vector.tensor_mul(n1[:], av[:, 0, 64:65], bv[:, 0, 64:65])
        nc.vector.tensor_mul(n2[:], av[:, 0, F2P + 64 : F2P + 65], bv[:, 0, F2P + 64 : F2P + 65])
        if conj_b:
            nc.vector.tensor_add(zv[:, 0, 64:65], n1[:], n2[:])
        else:
            nc.vector.tensor_sub(zv[:, 0, 64:65], n1[:], n2[:])
        # DC halving (f1=0, f2=0, re; im DC is exactly zero)
        nc.vector.tensor_scalar_mul(zcat[:, 0:1], zcat[:, 0:1], 0.5)

    # ---------------- corner turn + inverse rfft ---------------------------
    def turn_transposes(zcat, ht, J, tp_psum):
        """Transpose one hd-chunk's interleaved product into J columns."""
        for f1 in range(16):
            pt = tp_psum.tile([128, 128], f16, name=f"pt{f1 % 2}", tag=f"pt{f1 % 2}")
            nc.tensor.transpose(pt[:], zcat[:, f1 * 128 : (f1 + 1) * 128], ident[:])
            dsl = slice(f1 * 512 + ht * 128, f1 * 512 + ht * 128 + 128)
            if f1 % 2 == 0:
                nc.scalar.copy(J[:, dsl], pt[:])
            else:
                nc.vector.tensor_copy(J[:, dsl], pt[:])

    def inverse_tail(name, J, dst_big, pools, tail_unit=None):
        """J: [128 rows=(re f2 0..63 | im f2 0..63; f1=0: re 0..64 | im 1..63),
        16*512 cols (f1-major, hd)].  dst_big: [128, 4*2048] f16
        (hd-chunk-major), cols within each chunk in PERMUTED time order
        c = wp*128 + t2l*16 + t1  (true t = t1*128 + 8*wp + t2l)."""
        wk_pool, iv_psum, ct_pool = pools
        cbig = wk_pool.tile([128, 2 * NW * 512], f16, name=f"cbig_{name}")
        for f1 in range(16):
            psA = iv_psum.tile([128, 512], f32, name=f"psA{f1 % 2}", tag="psA")
            psB = iv_psum.tile([128, 512], f32, name=f"psB{f1 % 2}", tag="psB")
            jsl = slice(f1 * 512, (f1 + 1) * 512)
            ua_f = ua0_t if f1 == 0 else ua_t
            ub_f = ub0_t if f1 == 0 else ub_t
            nc.tensor.matmul(psA[:], ua_f[:], J[:, jsl], start=True, stop=True)
            nc.tensor.matmul(psB[:], ub_f[:], J[:, jsl], start=True, stop=True)
            nc.scalar.copy(cbig[:, f1 * 512 : (f1 + 1) * 512], psA[:])
            nc.vector.tensor_copy(cbig[:, 8192 + f1 * 512 : 8192 + (f1 + 1) * 512], psB[:])
        # stage 2: per wp, SBUF->SBUF corner turn + 2 accumulating matmuls/hc
        cview = cbig[:].rearrange("p (ri f1 d) -> p ri f1 d", ri=2, f1=16)
        for wp in range(NW):
            ct = ct_pool.tile([128, 1024], f16, name=f"ct{wp % 6}", tag=f"ct{wp % 6}")
            nc.sync.dma_start(
                out=ct[:, 0:512], in_=cview[8 * wp : 8 * wp + 8, 0]
            )
            nc.gpsimd.dma_start(
                out=ct[:, 512:1024], in_=cview[8 * wp : 8 * wp + 8, 1]
            )
            pr = iv_psum.tile([128, 512], f32, name=f"pr{wp % 2}", tag=f"pr{wp % 2}")
            for hc in range(4):
                nc.tensor.matmul(
                    pr[:, hc * 128 : (hc + 1) * 128], ct[:, hc * 128 : (hc + 1) * 128],
                    w2c[:, wp * 256 : wp * 256 + 128], start=True, stop=False,
                )
                nc.tensor.matmul(
                    pr[:, hc * 128 : (hc + 1) * 128], ct[:, 512 + hc * 128 : 512 + (hc + 1) * 128],
                    w2c[:, wp * 256 + 128 : wp * 256 + 256], start=False, stop=True,
                )
            dst = dst_big[:].rearrange("p (hc c) -> p hc c", hc=4)[
                :, :, wp * 128 : (wp + 1) * 128
            ]
            srcv = pr[:].rearrange("p (hc c) -> p hc c", hc=4)
            if wp % 2 == 0:
                nc.vector.tensor_copy(dst, srcv)
            else:
                nc.scalar.copy(dst, srcv)
            if tail_unit is not None:
                tail_unit(wp)

    # ---------------- Z = Qhf * conj(Khf) -> R -----------------------------
    rbufbig = big_pool.tile([128, 4 * L], f16, name="rbufbig", tag="big")
    with tc.tile_pool(name="zwork", bufs=1) as zw_pool, \
         tc.tile_pool(name="tppsum", bufs=1, space="PSUM") as tp_psum, \
         tc.tile_pool(name="ivpsum", bufs=1, space="PSUM") as iv_psum, \
         tc.tile_pool(name="ctpool", bufs=1) as ct_pool:
        J_r = zw_pool.tile([128, NW * 512], f16, name="J_r")
        for ht in range(4):
            zc = spS_pool.tile([128, 16 * 128], f16, name=f"zcat{ht}", tag="zsp", bufs=4)
            specmul(zc[:], proj["q"][ht][:], proj["k"][ht][:], True, zw_pool)
            turn_transposes(zc[:], ht, J_r, tp_psum)
        inverse_tail("r", J_r, rbufbig, (zw_pool, iv_psum, ct_pool))
    _pk_cm.__exit__(None, None, None)

    # ---------------- top-k + softmax + spike build ------------------------
    # S spectra slots (shared with the dead zcat tiles); the S time-domain
    # transposes are interleaved into the top-k loop so the PE has work
    # during the DVE-serial scans.
    spS = [
        spS_pool.tile([128, 16 * NC2], f16, name=f"sp_S_{rc}", tag="zsp", bufs=4)
        for rc in range(4)
    ]
    _st_cm = tc.tile_pool(name="strans", bufs=1)
    st_pool = _st_cm.__enter__()
    _stp_cm = tc.tile_pool(name="stpsum", bufs=1, space="PSUM")
    st_psum = _stp_cm.__enter__()
    # scratch cols ordered (j, hc, d) so the per-w load is a 3-dim DMA
    scratch = st_pool.tile([128, 4 * 16 * 128], f16, name="s_scr")
    svals = []
    with tc.tile_pool(name="topk", bufs=1) as tk_pool:
        for hc in range(4):
            sfx = hc % 2
            v8a = tk_pool.tile([128, 8], f16, name=f"v8a{hc}", tag=f"v8a{sfx}")
            v8b = tk_pool.tile([128, 8], f16, name=f"v8b{hc}", tag=f"v8b{sfx}")
            i8a = tk_pool.tile([128, 8], u32, name=f"i8a{hc}", tag=f"i8a{sfx}")
            i8b = tk_pool.tile([128, 8], u32, name=f"i8b{hc}", tag=f"i8b{sfx}")
            rscr = tk_pool.tile([128, L], f16, name=f"rscr{hc}", tag=f"rscr{sfx}")
            rview = rbufbig[:, hc * L : (hc + 1) * L]
            nc.vector.max(v8a[:], rview)
            nc.vector.max_index(i8a[:], v8a[:], rview)
            nc.vector.match_replace(rscr[:], v8a[:], rview, -60000.0)
            nc.vector.max(v8b[:], rscr[:])
            nc.vector.max_index(i8b[:], v8b[:], rscr[:])
            vals = tk_pool.tile([128, 16], f32, name=f"vals{hc}", tag=f"vals{sfx}")
            idxs = tk_pool.tile([128, 16], i32, name=f"idxs{hc}", tag=f"idxs{sfx}")
            nc.vector.tensor_copy(vals[:, 0:8], v8a[:])
            nc.vector.tensor_copy(vals[:, 8:16], v8b[:])
            nc.vector.tensor_copy(idxs[:, 0:8], i8a[:].bitcast(i32))
            nc.vector.tensor_copy(idxs[:, 8:16], i8b[:].bitcast(i32))
            # un-permute: c -> t = (c&15)*128 + (c>>7)*8 + ((c>>4)&7)
            tA = tk_pool.tile([128, 16], i32, name=f"tA{hc}", tag=f"tA{sfx}")
            tBv = tk_pool.tile([128, 16], i32, name=f"tB{hc}", tag=f"tB{sfx}")
            nc.vector.tensor_scalar(
                out=tA[:], in0=idxs[:], scalar1=15, scalar2=7,
                op0=ALU.bitwise_and, op1=ALU.logical_shift_left,
            )
            nc.vector.tensor_scalar(
                out=tBv[:], in0=idxs[:], scalar1=7, scalar2=3,
                op0=ALU.logical_shift_right, op1=ALU.logical_shift_left,
            )
            nc.vector.tensor_add(tA[:], tA[:], tBv[:])
            nc.vector.tensor_scalar(
                out=tBv[:], in0=idxs[:], scalar1=4, scalar2=7,
                op0=ALU.logical_shift_right, op1=ALU.bitwise_and,
            )
            nc.vector.tensor_add(tA[:], tA[:], tBv[:])
            # softmax over cols 0..14
            negmax = tk_pool.tile([128, 1], f32, name=f"negmax{hc}", tag=f"negmax{sfx}")
            nc.vector.tensor_scalar_mul(negmax[:], vals[:, 0:1], -1.0)
            e15 = tk_pool.tile([128, TOPK], f32, name=f"e15{hc}", tag=f"e15{sfx}")
            nc.scalar.activation(e15[:], vals[:, 0:TOPK], AF.Exp, bias=negmax[:], scale=1.0)
            s15 = tk_pool.tile([128, 1], f32, name=f"s15{hc}", tag=f"s15{sfx}")
            nc.vector.tensor_reduce(s15[:], e15[:], axis=AX.X, op=ALU.add)
            r15 = tk_pool.tile([128, 1], f32, name=f"r15{hc}", tag=f"r15{sfx}")
            nc.vector.reciprocal(r15[:], s15[:])
            wts = tk_pool.tile([128, 16], f16, name=f"wts{hc}", tag=f"wts{sfx}")
            nc.vector.memset(wts[:], 0.0)
            w15f = tk_pool.tile([128, TOPK], f32, name=f"w15f{hc}", tag=f"w15f{sfx}")
            nc.vector.tensor_scalar_mul(w15f[:], e15[:], r15[:])
            nc.vector.tensor_copy(wts[:, 0:TOPK], w15f[:])
            # pos = (2048 - t) & 2047 ; split halves with -1 padding
            pos = tk_pool.tile([128, 16], i32, name=f"pos{hc}", tag=f"pos{sfx}")
            nc.vector.tensor_scalar(out=pos[:], in0=tA[:], scalar1=-1, scalar2=2048, op0=ALU.mult, op1=ALU.add)
            nc.vector.tensor_scalar(out=pos[:], in0=pos[:], scalar1=2047, scalar2=None, op0=ALU.bitwise_and)
            mlt = tk_pool.tile([128, 16], i32, name=f"mlt{hc}", tag=f"mlt{sfx}")
            nc.vector.tensor_scalar(out=mlt[:], in0=pos[:], scalar1=1024, scalar2=None, op0=ALU.is_lt)
            mge = tk_pool.tile([128, 16], i32, name=f"mge{hc}", tag=f"mge{sfx}")
            nc.vector.tensor_scalar(out=mge[:], in0=pos[:], scalar1=1024, scalar2=None, op0=ALU.is_ge)
            # idx0 = pos*mlt + mlt - 1 ; idx1 = (pos-1024)*mge + mge - 1
            t0 = tk_pool.tile([128, 16], i32, name=f"t0{hc}", tag=f"t0{sfx}")
            nc.vector.tensor_mul(t0[:], pos[:], mlt[:])
            nc.vector.tensor_add(t0[:], t0[:], mlt[:])
            nc.vector.tensor_scalar_add(t0[:], t0[:], -1)
            t1b = tk_pool.tile([128, 16], i32, name=f"t1b{hc}", tag=f"t1b{sfx}")
            nc.vector.tensor_scalar_add(t1b[:], pos[:], -1024)
            nc.vector.tensor_mul(t1b[:], t1b[:], mge[:])
            nc.vector.tensor_add(t1b[:], t1b[:], mge[:])
            nc.vector.tensor_scalar_add(t1b[:], t1b[:], -1)
            ix0 = tk_pool.tile([128, 16], i16, name=f"ix0{hc}", tag=f"ix0{sfx}")
            ix1 = tk_pool.tile([128, 16], i16, name=f"ix1{hc}", tag=f"ix1{sfx}")
            nc.vector.tensor_copy(ix0[:], t0[:])
            nc.vector.tensor_copy(ix1[:], t1b[:])
            # build S rows for this hd chunk: [128, 2048] fp16
            s_sb = svrb_pool.tile([128, L], f16, name=f"s_sb{hc}")
            nc.gpsimd.local_scatter(s_sb[:, 0:1024], wts[:], ix0[:], 128, 1024, 16)
            nc.gpsimd.local_scatter(s_sb[:, 1024:2048], wts[:], ix1[:], 128, 1024, 16)
            svals.append(s_sb)
            for j in range(16):
                pt = st_psum.tile([128, 128], f16, name=f"spt{j % 2}", tag=f"spt{j % 2}")
                nc.tensor.transpose(pt[:], s_sb[:, j * 128 : (j + 1) * 128], ident[:])
                if j % 2 == 0:
                    nc.scalar.copy(scratch[:, (j * 4 + hc) * 128 : (j * 4 + hc) * 128 + 128], pt[:])
                else:
                    nc.vector.tensor_copy(scratch[:, (j * 4 + hc) * 128 : (j * 4 + hc) * 128 + 128], pt[:])
    _stp_cm.__exit__(None, None, None)

    # ---------------- S rfft -----------------------------------------------
    if True:
        with tc.tile_pool(name="xas", bufs=1) as xas_pool, \
             tc.tile_pool(name="sfwork", bufs=1) as sw_pool, \
             tc.tile_pool(name="sfpsum", bufs=1, space="PSUM") as sf_psum:
            xtilesS = {}

            def get_xviewS(w):
                for ww in (w, w + 1, w + 2, w + 3):
                    if ww in xtilesS or ww >= NW:
                        continue
                    xa = xas_pool.tile(
                        [128, D], f16, name=f"xaS{ww}", tag="xaS", bufs=4
                    )
                    nc.sync.dma_start(
                        out=xa[:],
                        in_=scratch[:].rearrange("p (j c) -> p j c", j=16)[
                            8 * ww : 8 * ww + 8
                        ],
                    )
                    xtilesS[ww] = xa
                return xtilesS[w][:]

            bbuf_S = big_pool.tile([128, 2 * NW * 512], f16, name="bbuf_S", tag="big")
            for s in stageA_steps("S", get_xviewS, m_re16, m_im16, sw_pool, sf_psum, bbuf_S):
                s()
            for s in stageB_steps("S", spS, bbuf_S, sf_psum):
                s()
    _st_cm.__exit__(None, None, None)
    _svrb_cm.__exit__(None, None, None)

    # ---------------- Y = Qhf * Sf, inverse, output proj -------------------
    _ag_cm = tc.tile_pool(name="aggp", bufs=1)
    ag_pool = _ag_cm.__enter__()
    aggbig = big_pool.tile([128, 4 * L], f16, name="aggbig", tag="big")
    with tc.tile_pool(name="ywork", bufs=1) as yw_pool, \
         tc.tile_pool(name="typsum", bufs=1, space="PSUM") as ty_psum, \
         tc.tile_pool(name="ivpsumy", bufs=1, space="PSUM") as iv_psumy, \
         tc.tile_pool(name="ctpooly", bufs=1) as ct_pooly, \
         tc.tile_pool(name="outp", bufs=2) as out_pool, \
         tc.tile_pool(name="outpsum", bufs=1, space="PSUM") as out_psum:
        outv = out_ext[:].rearrange(
            "(t1 wl t2l) d -> wl t2l t1 d", t1=16, wl=16, t2l=8
        )

        def out_unit(j):
            # out-proj column block j depends only on stage-2 chunk wp=j
            po = out_psum.tile([128, D], f32, name=f"po{j % 2}", tag=f"po{j % 2}")
            for hc in range(4):
                nc.tensor.matmul(
                    po[:],
                    aggbig[:, hc * L + j * 128 : hc * L + (j + 1) * 128],
                    woc[:, hc * D : (hc + 1) * D],
                    start=(hc == 0),
                    stop=(hc == 3),
                )
            ot = out_pool.tile([128, D], f32, name=f"ot{j % 2}", tag=f"ot{j % 2}")
            nc.vector.tensor_add(ot[:], po[:], bo_sb[:])
            nc.sync.dma_start(out=outv[j], in_=ot[:])

        J_y = yw_pool.tile([128, NW * 512], f16, name="J_y")
        for ht in range(4):
            yc = yw_pool.tile([128, 16 * 128], f16, name=f"ycat{ht}", tag=f"ycat{ht}")
            specmul(yc[:], proj["q"][ht][:], spS[ht][:], False, yw_pool)
            turn_transposes(yc[:], ht, J_y, ty_psum)
        inverse_tail("y", J_y, aggbig, (yw_pool, iv_psumy, ct_pooly))
        for j in range(NW):
            out_unit(j)

    _ag_cm.__exit__(None, None, None)
    _pk2 = None  # placeholder
    _spS_cm.__exit__(None, None, None)
    _phq_cm.__exit__(None, None, None)
    _big_cm.__exit__(None, None, None)


@functools.lru_cache(maxsize=1)
def _get_built():
    return _build()


def build_base_inputs(inputs):
    cs = _consts()
    return {
        "wq16": np.asarray(inputs["Wq"], np.float16),
        "wk16": np.asarray(inputs["Wk"], np.float16),
        "wo16": np.asarray(inputs["Wo"], np.float16),
        "bq2": np.asarray(inputs["bq"], np.float32).reshape(4, 128),
        "bk2": np.asarray(inputs["bk"], np.float32).reshape(4, 128),
        "borep": np.broadcast_to(
            np.asarray(inputs["bo"], np.float32), (128, D)
        ).copy(),
        "mreq16": cs["mreq16"], "mimq16": cs["mimq16"],
        "mre16": cs["mre16"], "mim16": cs["mim16"],
        "vcat": cs["vcat"], "smallc": cs["smallc"],
        "w216": cs["w216"],
    }


def kernel(**inputs):
    q = np.asarray(inputs["q"], np.float32)
    k = np.asarray(inputs["k"], np.float32)
    nc, par = _get_built()
    base = build_base_inputs(inputs)
    in_maps = [dict(base, q=q[b], k=k[b]) for b in range(NCORES)]
    res = run_bass_kernel_spmd(nc, in_maps, list(range(NCORES)))
    out = np.stack([res.results[b]["out"] for b in range(NCORES)], axis=0)
    return out.astype(np.float32)


if __name__ == "__main__":
    import reference  # noqa: F401

    ins = {k: np.asarray(v) for k, v in reference.setup_inputs().items()}
    got = kernel(**ins)
    exp = np.asarray(reference.reference(**ins))
    rel = np.linalg.norm(got - exp) / np.linalg.norm(exp)
    print("rel err:", rel)


# revision 53
# speedup vs baseline: 1.0422x; 1.0422x over previous
"""AutoCorrelation (Autoformer) Trainium2 kernel.

Per (batch, head, depth-row) r of the projected series qt, kt (length L=2048):
R = irfft(rfft(qt) * conj(rfft(kt))); top-15 lags of R; softmax over those
R-values; out = (sum_i w_i * roll(qt, -idx_i)) @ Wo + bo.

Strategy (one batch per core, 8 cores):
  - rfft via two matmul stages on the RAW q/k (FFT commutes with the feature
    projection), so the Wq/Wk projection happens in the frequency domain and
    no input transposes are needed.
  - aggregation in frequency domain: sum_i w_i*roll(qt,-m_i) = irfft(Qf * Sf),
    S a sparse spike train built with gpsimd local_scatter.
  - top-k via DVE max/max_index/match_replace; softmax via ACT exp.
  - inverse rfft: packed 128-deep stage-1 matmuls ([re|im] joint operand),
    SBUF-resident corner turn, permuted time order (unscrambled only in the
    final output DMA / top-k index arithmetic).
"""

import functools
import math
import os
import sys

import numpy as np

sys.path.insert(0, "/opt/trn_rl_repo")

import concourse.bacc as bacc  # noqa: E402
import concourse.mybir as mybir  # noqa: E402
from concourse.bass_utils import run_bass_kernel_spmd  # noqa: E402
from concourse.tile import TileContext  # noqa: E402

B, L, D, H = 8, 2048, 512, 8
TOPK = int(2 * math.log(L))  # 15
NW = 16
F2 = 65   # rfft second-factor range: f = f1 + 16*f2, f1<16, f2<65
F2P = 66  # padded (col 65 zero) so [re|im] blocks are 4B-aligned
NC2 = 2 * F2P  # 132 cols per f1 in a spectrum tile
NCORES = 8

f32 = mybir.dt.float32
f32r = mybir.dt.float32r
f16 = mybir.dt.float16
i16 = mybir.dt.int16
i32 = mybir.dt.int32
u32 = mybir.dt.uint32
AF = mybir.ActivationFunctionType
ALU = mybir.AluOpType
AX = mybir.AxisListType


@functools.lru_cache(maxsize=1)
def _consts():
    c = {}
    # q/k variant: partition p = t1*8 + t2l (t1-major, matches XA dma dim order)
    Mre = np.zeros((NW, 128, 128), np.float64)
    Mim = np.zeros((NW, 128, 128), np.float64)
    # S variant: partition p = t2l*16 + t1 (t2l-major, matches scratch relayout)
    MreS = np.zeros((NW, 128, 128), np.float64)
    MimS = np.zeros((NW, 128, 128), np.float64)
    for w in range(NW):
        for t2l in range(8):
            for t1 in range(16):
                t = t1 * 128 + 8 * w + t2l
                f1v = np.arange(16)
                ang = 2 * np.pi * f1v * t / L
                m = t2l * 16 + f1v
                Mre[w, t1 * 8 + t2l, m] = np.cos(ang)
                Mim[w, t1 * 8 + t2l, m] = -np.sin(ang)
                MreS[w, t2l * 16 + t1, m] = np.cos(ang)
                MimS[w, t2l * 16 + t1, m] = -np.sin(ang)
    c["mre"] = Mre.reshape(NW * 128, 128).astype(np.float32)
    c["mim"] = Mim.reshape(NW * 128, 128).astype(np.float32)
    c["mre16"] = MreS.reshape(NW * 128, 128).astype(np.float16)
    c["mim16"] = MimS.reshape(NW * 128, 128).astype(np.float16)
    # forward second stage, padded to F2P (col 65 of each quadrant = 0)
    t2 = np.arange(128)[:, None]
    f2 = np.arange(F2P)[None, :]
    Vre = np.cos(2 * np.pi * t2 * f2 / 128)
    Vim = -np.sin(2 * np.pi * t2 * f2 / 128)
    Vre[:, F2:] = 0.0
    Vim[:, F2:] = 0.0
    c["vcat"] = np.concatenate([Vre, Vim, -Vim, Vre], axis=1).astype(np.float16)
    # inverse stage-1 packed stationaries: J rows = [re f2 0..63 | im f2 0..63]
    f2c = np.arange(64)[:, None]
    t2p = np.arange(128)[None, :]
    Ure = (2.0 / L) * np.cos(2 * np.pi * f2c * t2p / 128)
    Uim = (2.0 / L) * np.sin(2 * np.pi * f2c * t2p / 128)
    c["ua"] = np.concatenate([Ure, -Uim], axis=0).astype(np.float16)  # -> C_re
    c["ub"] = np.concatenate([Uim, Ure], axis=0).astype(np.float16)   # -> C_im
    # f1=0 variants: J0 rows = [re f2 0..64 (64=Nyquist) | im f2 1..63].
    # Nyquist row (f=1024): (2/L)*cos(pi*t2) with the 0.5 halving folded in.
    u64row = (1.0 / L) * np.cos(np.pi * t2p)
    c["ua0"] = np.concatenate([Ure, u64row, -Uim[1:64]], axis=0).astype(np.float16)
    c["ub0"] = np.concatenate([Uim, np.zeros_like(u64row), Ure[1:64]], axis=0).astype(
        np.float16
    )
    W2 = np.zeros((NW, 128, 256), np.float64)
    for wp in range(NW):
        for t2l in range(8):
            for f1 in range(16):
                p = t2l * 16 + f1
                t1pp = np.arange(16)
                t = 128 * t1pp + 8 * wp + t2l
                ang = 2 * np.pi * f1 * t / L
                W2[wp, p, t2l * 16 + t1pp] = np.cos(ang)
                W2[wp, p, 128 + t2l * 16 + t1pp] = -np.sin(ang)
    c["w216"] = W2.reshape(NW * 128, 256).astype(np.float16)
    # packed small consts: [ident | ua | ub | ua0 | ub0] as one [128, 640] f16
    c["smallc"] = np.concatenate(
        [np.eye(128, dtype=np.float16), c["ua"], c["ub"], c["ua0"], c["ub0"]],
        axis=1,
    )
    # packed S-fft stage-A matrices: [mre16 cols | mim16 cols] -> [128, 4096]
    return c


def _build(dbg=()):
    nc = bacc.Bacc("TRN2", target_bir_lowering=False, num_devices=NCORES)

    par = {}

    def dram(name, shape, dt, out=False):
        par[name] = nc.declare_dram_parameter(name, list(shape), dt, isOutput=out)
        return par[name]

    q_in = dram("q", [L, D], f32r)
    k_in = dram("k", [L, D], f32r)
    mre_in = dram("mre", [NW * 128, 128], f32r)
    mim_in = dram("mim", [NW * 128, 128], f32r)
    wq_in = dram("wq16", [D, D], f16)
    wk_in = dram("wk16", [D, D], f16)
    wo_in = dram("wo16", [D, D], f16)
    bq_in = dram("bq2", [4, 128], f32)
    bk_in = dram("bk2", [4, 128], f32)
    bo_in = dram("borep", [128, D], f32)
    mre16_in = dram("mre16", [NW * 128, 128], f16)
    mim16_in = dram("mim16", [NW * 128, 128], f16)
    v_in = dram("vcat", [128, 4 * F2P], f16)
    smallc_in = dram("smallc", [128, 640], f16)
    w2_in = dram("w216", [NW * 128, 256], f16)
    out_ext = dram("out", [L, D], f32, out=True)

    with TileContext(nc) as tc:
        with tc.tile_pool(name="consts", bufs=1) as consts:
            vcat = consts.tile([128, 4 * F2P], f16, name="vcat")
            wq16 = consts.tile([128, 4 * D], f16, name="wq16")
            wk16 = consts.tile([128, 4 * D], f16, name="wk16")
            bq_sb = consts.tile([128, 4], f32, name="bq_sb")
            bk_sb = consts.tile([128, 4], f32, name="bk_sb")
            smallc = consts.tile([128, 640], f16, name="smallc")
            ident = smallc[:, 0:128]
            ua_t = smallc[:, 128:256]
            ub_t = smallc[:, 256:384]
            ua0_t = smallc[:, 384:512]
            ub0_t = smallc[:, 512:640]
            m_re16 = consts.tile([128, NW * 128], f16, name="m_re16")
            m_im16 = consts.tile([128, NW * 128], f16, name="m_im16")
            w2c = consts.tile([128, NW * 256], f16, name="w2c")
            woc = consts.tile([128, 4 * D], f16, name="woc")
            bo_sb = consts.tile([128, D], f32, name="bo_sb")

            def load_consts():
                # early (scalar queue): only what the q/k fft + proj need soon
                nc.scalar.dma_start(out=vcat[:], in_=v_in[:])
                nc.scalar.dma_start(out=smallc[:], in_=smallc_in[:])
                nc.scalar.dma_start(
                    out=wq16[:], in_=wq_in[:].rearrange("(fc p) c -> p fc c", fc=4)
                )
                nc.scalar.dma_start(
                    out=wk16[:], in_=wk_in[:].rearrange("(fc p) c -> p fc c", fc=4)
                )
                nc.scalar.dma_start(out=bq_sb[:], in_=bq_in[:].rearrange("c p -> p c"))
                nc.scalar.dma_start(out=bk_sb[:], in_=bk_in[:].rearrange("c p -> p c"))

            def load_consts_late():
                # bulk consts for the back half, issued once the q fft is
                # underway (scalar queue is free again after the A(q) evacs)
                nc.scalar.dma_start(
                    out=w2c[:], in_=w2_in[:].rearrange("(w p) c -> p w c", w=NW)
                )
                nc.scalar.dma_start(
                    out=m_re16[:], in_=mre16_in[:].rearrange("(w p) c -> p w c", w=NW)
                )
                nc.scalar.dma_start(
                    out=m_im16[:], in_=mim16_in[:].rearrange("(w p) c -> p w c", w=NW)
                )
                nc.scalar.dma_start(
                    out=woc[:], in_=wo_in[:].rearrange("(fc p) c -> p fc c", fc=4)
                )
                nc.scalar.dma_start(out=bo_sb[:], in_=bo_in[:])

            _pipeline(nc, tc, locals())

    nc.compile()
    return nc, par


def _pipeline(nc, tc, E):
    q_in, k_in = E["q_in"], E["k_in"]
    mre_in, mim_in = E["mre_in"], E["mim_in"]
    out_ext = E["out_ext"]
    m_re16, m_im16 = E["m_re16"], E["m_im16"]
    vcat, w2c = E["vcat"], E["w2c"]
    ua_t, ub_t = E["ua_t"], E["ub_t"]
    ua0_t, ub0_t = E["ua0_t"], E["ub0_t"]
    wq16, wk16, woc = E["wq16"], E["wk16"], E["woc"]
    bq_sb, bk_sb, bo_sb = E["bq_sb"], E["bk_sb"], E["bo_sb"]
    load_consts = E["load_consts"]
    load_consts_late = E["load_consts_late"]
    ident = E["ident"]

    # ---------------- forward rfft (shared by q, k, S) ---------------------
    # Emitted as per-step closures so independent work can be interleaved in
    # program order (engines execute their queues strictly in order).
    def stageA_steps(name, get_xview, mr, mi, work_pool, psum_pool, bbuf):
        bview = bbuf[:].rearrange("p (ri f1 d) -> p ri f1 d", ri=2, f1=16)

        pending = []

        def flush(lag):
            # Emit relayout DMAs a couple of iterations late so their evac
            # dependencies are long satisfied by the time they reach the
            # queue head — a waiting DMA stalls everything behind it on its
            # in-order queue.
            while len(pending) > lag:
                w, ev = pending.pop(0)
                nc.scalar.dma_start(out=bview[8 * w : 8 * w + 8, 0], in_=ev[:, 0:512])
                nc.gpsimd.dma_start(out=bview[8 * w : 8 * w + 8, 1], in_=ev[:, 512:1024])

        def step(w):
            xv = get_xview(w)
            ev = work_pool.tile(
                [128, 1024], f16, name=f"ev{w % 5}", tag=f"ev{w % 5}"
            )
            par_ = psum_pool.tile([128, 512], f32, name=f"par{w % 3}", tag=f"par{w % 3}")
            pai = psum_pool.tile([128, 512], f32, name=f"pai{w % 3}", tag=f"pai{w % 3}")
            wsl = slice(w * 128, (w + 1) * 128)
            nc.tensor.matmul(par_[:], mr[:, wsl], xv, start=True, stop=True)
            nc.tensor.matmul(pai[:], mi[:, wsl], xv, start=True, stop=True)
            nc.scalar.copy(ev[:, 0:512], par_[:])
            nc.vector.tensor_copy(ev[:, 512:1024], pai[:])
            pending.append((w, ev))
            flush(2)
            if w == NW - 1:
                flush(0)

        return [lambda w=w: step(w) for w in range(NW)]

    def stageB_steps(name, sp, bbuf, psum_pool):
        def step(rc, f1):
            pb = psum_pool.tile([128, NC2], f32, name=f"pb{f1 % 2}", tag=f"pb{f1 % 2}")
            bsl = slice(f1 * D + rc * 128, f1 * D + rc * 128 + 128)
            isl = slice(NW * 512 + f1 * D + rc * 128, NW * 512 + f1 * D + rc * 128 + 128)
            nc.tensor.matmul(pb[:], bbuf[:, bsl], vcat[:, :NC2], start=True, stop=False)
            nc.tensor.matmul(pb[:], bbuf[:, isl], vcat[:, NC2:], start=False, stop=True)
            if (rc * 16 + f1) % 2 == 0:
                nc.scalar.copy(sp[rc][:, f1 * NC2 : (f1 + 1) * NC2], pb[:])
            else:
                nc.vector.tensor_copy(sp[rc][:, f1 * NC2 : (f1 + 1) * NC2], pb[:])

        return [lambda rc=rc, f1=f1: step(rc, f1) for rc in range(4) for f1 in range(16)]

    # ---------------- long-lived pools (opened in LIFO close order) --------
    _big_cm = tc.tile_pool(name="bigpool", bufs=1)
    big_pool = _big_cm.__enter__()
    _phq_cm = tc.tile_pool(name="projq", bufs=1)
    phq_pool = _phq_cm.__enter__()
    _spS_cm = tc.tile_pool(name="spSp", bufs=1)
    spS_pool = _spS_cm.__enter__()
    _svrb_cm = tc.tile_pool(name="svrb", bufs=1)
    svrb_pool = _svrb_cm.__enter__()
    _pk_cm = tc.tile_pool(name="projk", bufs=1)
    pk_pool = _pk_cm.__enter__()
    _raw_cm = tc.tile_pool(name="rawsp", bufs=1)
    rawsp_pool = _raw_cm.__enter__()

    # ---------------- q/k loads + fft + projection, interleaved ------------
    ncols = 16 * NC2  # 2112
    spect = {}
    proj = {}
    with tc.tile_pool(name="bigc", bufs=1) as bigc, \
         tc.tile_pool(name="xa", bufs=1) as xa_pool, \
         tc.tile_pool(name="fftwork", bufs=1) as work_pool, \
         tc.tile_pool(name="projtmp", bufs=1) as pj_tmp, \
         tc.tile_pool(name="fftpsum", bufs=1, space="PSUM") as psum_pool:
        m_re = bigc.tile([128, NW * 128], f32r, name="m_re")
        m_im = bigc.tile([128, NW * 128], f32r, name="m_im")
        nc.scalar.dma_start(
            out=m_re[:], in_=mre_in[:].rearrange("(w p) c -> p w c", w=NW)
        )
        nc.scalar.dma_start(
            out=m_im[:], in_=mim_in[:].rearrange("(w p) c -> p w c", w=NW)
        )
        load_consts()

        def make_get_xview(nm, src):
            srcv = src[:].rearrange(
                "(t1 wl t2l) d -> wl t1 t2l d", t1=16, wl=16, t2l=8
            )
            tiles = {}

            def load(w):
                if w in tiles or w >= NW:
                    return
                xa = xa_pool.tile(
                    [128, D], f32r, name=f"xa{nm}{w}", tag="xa", bufs=6
                )
                nc.sync.dma_start(out=xa[:], in_=srcv[w])
                tiles[w] = xa

            def get(w):
                for ww in (w, w + 1, w + 2, w + 3, w + 4, w + 5):
                    load(ww)
                return tiles[w][:]

            return get

        def proj_units(nm, wmat, bias):
            dstp = phq_pool if nm == "q" else pk_pool
            ph = [dstp.tile([128, ncols], f16, name=f"ph_{nm}_{ht}") for ht in range(4)]
            proj[nm] = ph

            def unit(ht, c0):
                cw = min(512, ncols - c0)
                pp = psum_pool.tile(
                    [128, 512], f32, name=f"pp{ht % 2}", tag=f"pb{ht % 2}"
                )
                for fc in range(4):
                    nc.tensor.matmul(
                        pp[:, :cw],
                        wmat[:, fc * D + ht * 128 : fc * D + ht * 128 + 128],
                        spect[nm][fc][:, c0 : c0 + cw],
                        start=(fc == 0),
                        stop=(fc == 3),
                    )
                if c0 == 0:
                    # bias: rfft(b*ones)[f=0] = L*b -> col 0 (f1=0, re, f2=0)
                    bb = pj_tmp.tile([128, 1], f32, name=f"bb_{nm}_{ht}", tag="bbias")
                    nc.vector.tensor_scalar_mul(bb[:], bias[:, ht : ht + 1], float(L))
                    nc.vector.tensor_tensor(
                        out=pp[:, 0:1], in0=pp[:, 0:1], in1=bb[:], op=ALU.add
                    )
                nc.scalar.copy(ph[ht][:, c0 : c0 + cw], pp[:, :cw])

            return [
                lambda ht=ht, c0=c0: unit(ht, c0)
                for ht in range(4)
                for c0 in range(0, ncols, 512)
            ]

        for nm in ("q", "k"):
            spect[nm] = [
                rawsp_pool.tile([128, 16 * NC2], f16, name=f"sp_{nm}_{rc}")
                for rc in range(4)
            ]
        bbuf_q = big_pool.tile([128, 2 * NW * 512], f16, name="bbuf_q", tag="big")
        bbuf_k = big_pool.tile([128, 2 * NW * 512], f16, name="bbuf_k", tag="big")
        sA_q = stageA_steps("q", make_get_xview("q", q_in), m_re, m_im,
                            work_pool, psum_pool, bbuf_q)
        sA_k = stageA_steps("k", make_get_xview("k", k_in), m_re, m_im,
                            work_pool, psum_pool, bbuf_k)
        sB_q = stageB_steps("q", spect["q"], bbuf_q, psum_pool)
        sB_k = stageB_steps("k", spect["k"], bbuf_k, psum_pool)
        pU_q = proj_units("q", wq16, bq_sb)
        pU_k = proj_units("k", wk16, bk_sb)

        for s in sA_q:
            s()
        load_consts_late()
        for s in sB_q:
            s()
        # proj(q) interleaved with A(k): the A(k) chain trickles through the
        # evac/DMA engines while proj keeps the PE busy.
        ak = 0
        for u in pU_q:
            if ak < NW:
                sA_k[ak]()
                ak += 1
            u()
        while ak < NW:
            sA_k[ak]()
            ak += 1
        for s in sB_k:
            s()
        for u in pU_k:
            u()
    _raw_cm.__exit__(None, None, None)

    # ---------------- fused complex product --------------------------------
    def specmul(zcat, a, b, conj_b, mul_pool):
        """a,b: [128, 16*132] f16 spectra. zcat: [128, 16*128] f16, per f1:
        cols [0:64] = Z_re f2 0..63, cols [64:128] = Z_im f2 0..63.
        For f1=0, col 64 is overwritten with Z_re f2=64 (Nyquist); the true
        im f2=0 there is exactly zero. DC (col 0) halved."""
        av = a.rearrange("p (f1 c) -> p f1 c", f1=16)
        bv = b.rearrange("p (f1 c) -> p f1 c", f1=16)
        ar, ai = av[:, :, 0:64], av[:, :, F2P : F2P + 64]
        br, bi = bv[:, :, 0:64], bv[:, :, F2P : F2P + 64]
        zv = zcat.rearrange("p (f1 c) -> p f1 c", f1=16)
        dre, dim = zv[:, :, 0:64], zv[:, :, 64:128]
        t1 = mul_pool.tile([128, 16 * 64], f16, name="smt1", tag="smt1")
        t2 = mul_pool.tile([128, 16 * 64], f16, name="smt2", tag="smt2")
        t1v = t1[:].rearrange("p (f1 c) -> p f1 c", f1=16)
        t2v = t2[:].rearrange("p (f1 c) -> p f1 c", f1=16)
        nc.vector.tensor_mul(t1v, ar, br)
        nc.vector.tensor_mul(t2v, ai, bi)
        if conj_b:
            nc.vector.tensor_add(dre, t1v, t2v)
        else:
            nc.vector.tensor_sub(dre, t1v, t2v)
        nc.vector.tensor_mul(t1v, ai, br)
        nc.vector.tensor_mul(t2v, ar, bi)
        if conj_b:
            nc.vector.tensor_sub(dim, t1v, t2v)
        else:
            nc.vector.tensor_add(dim, t1v, t2v)
        # Nyquist (f1=0, f2=64, re) -> col 64 of the f1=0 block
        n1 = mul_pool.tile([128, 1], f16, name="smn1", tag="smn1")
        n2 = mul_pool.tile([128, 1], f16, name="smn2", tag="smn2")
        nc.vector.tensor_mul(n1[:], av[:, 0, 64:65], bv[:, 0, 64:65])
        nc.vector.tensor_mul(n2[:], av[:, 0, F2P + 64 : F2P + 65], bv[:, 0, F2P + 64 : F2P + 65])
        if conj_b:
            nc.vector.tensor_add(zv[:, 0, 64:65], n1[:], n2[:])
        else:
            nc.vector.tensor_sub(zv[:, 0, 64:65], n1[:], n2[:])
        # DC halving (f1=0, f2=0, re; im DC is exactly zero)
        nc.vector.tensor_scalar_mul(zcat[:, 0:1], zcat[:, 0:1], 0.5)

    # ---------------- corner turn + inverse rfft ---------------------------
    def turn_transposes(zcat, ht, J, tp_psum):
        """Transpose one hd-chunk's interleaved product into J columns."""
        for f1 in range(16):
            pt = tp_psum.tile([128, 128], f16, name=f"pt{f1 % 2}", tag=f"pt{f1 % 2}")
            nc.tensor.transpose(pt[:], zcat[:, f1 * 128 : (f1 + 1) * 128], ident[:])
            dsl = slice(f1 * 512 + ht * 128, f1 * 512 + ht * 128 + 128)
            if f1 % 2 == 0:
                nc.scalar.copy(J[:, dsl], pt[:])
            else:
                nc.vector.tensor_copy(J[:, dsl], pt[:])

    def inverse_tail(name, J, dst_big, pools, tail_unit=None):
        """J: [128 rows=(re f2 0..63 | im f2 0..63; f1=0: re 0..64 | im 1..63),
        16*512 cols (f1-major, hd)].  dst_big: [128, 4*2048] f16
        (hd-chunk-major), cols within each chunk in PERMUTED time order
        c = wp*128 + t2l*16 + t1  (true t = t1*128 + 8*wp + t2l)."""
        wk_pool, iv_psum, ct_pool = pools
        cbig = wk_pool.tile([128, 2 * NW * 512], f16, name=f"cbig_{name}")
        for f1 in range(16):
            psA = iv_psum.tile([128, 512], f32, name=f"psA{f1 % 2}", tag="psA")
            psB = iv_psum.tile([128, 512], f32, name=f"psB{f1 % 2}", tag="psB")
            jsl = slice(f1 * 512, (f1 + 1) * 512)
            ua_f = ua0_t if f1 == 0 else ua_t
            ub_f = ub0_t if f1 == 0 else ub_t
            nc.tensor.matmul(psA[:], ua_f[:], J[:, jsl], start=True, stop=True)
            nc.tensor.matmul(psB[:], ub_f[:], J[:, jsl], start=True, stop=True)
            nc.scalar.copy(cbig[:, f1 * 512 : (f1 + 1) * 512], psA[:])
            nc.vector.tensor_copy(cbig[:, 8192 + f1 * 512 : 8192 + (f1 + 1) * 512], psB[:])
        # stage 2: per wp, SBUF->SBUF corner turn + 2 accumulating matmuls/hc
        cview = cbig[:].rearrange("p (ri f1 d) -> p ri f1 d", ri=2, f1=16)
        for wp in range(NW):
            ct = ct_pool.tile([128, 1024], f16, name=f"ct{wp % 6}", tag=f"ct{wp % 6}")
            nc.sync.dma_start(
                out=ct[:, 0:512], in_=cview[8 * wp : 8 * wp + 8, 0]
            )
            nc.gpsimd.dma_start(
                out=ct[:, 512:1024], in_=cview[8 * wp : 8 * wp + 8, 1]
            )
            pr = iv_psum.tile([128, 512], f32, name=f"pr{wp % 2}", tag=f"pr{wp % 2}")
            for hc in range(4):
                nc.tensor.matmul(
                    pr[:, hc * 128 : (hc + 1) * 128], ct[:, hc * 128 : (hc + 1) * 128],
                    w2c[:, wp * 256 : wp * 256 + 128], start=True, stop=False,
                )
                nc.tensor.matmul(
                    pr[:, hc * 128 : (hc + 1) * 128], ct[:, 512 + hc * 128 : 512 + (hc + 1) * 128],
                    w2c[:, wp * 256 + 128 : wp * 256 + 256], start=False, stop=True,
                )
            dst = dst_big[:].rearrange("p (hc c) -> p hc c", hc=4)[
                :, :, wp * 128 : (wp + 1) * 128
            ]
            srcv = pr[:].rearrange("p (hc c) -> p hc c", hc=4)
            if wp % 2 == 0:
                nc.vector.tensor_copy(dst, srcv)
            else:
                nc.scalar.copy(dst, srcv)
            if tail_unit is not None:
                tail_unit(wp)

    # ---------------- Z = Qhf * conj(Khf) -> R -----------------------------
    rbufbig = big_pool.tile([128, 4 * L], f16, name="rbufbig", tag="big")
    with tc.tile_pool(name="zwork", bufs=1) as zw_pool, \
         tc.tile_pool(name="tppsum", bufs=1, space="PSUM") as tp_psum, \
         tc.tile_pool(name="ivpsum", bufs=1, space="PSUM") as iv_psum, \
         tc.tile_pool(name="ctpool", bufs=1) as ct_pool:
        J_r = zw_pool.tile([128, NW * 512], f16, name="J_r")
        for ht in range(4):
            zc = spS_pool.tile([128, 16 * 128], f16, name=f"zcat{ht}", tag="zsp", bufs=4)
            specmul(zc[:], proj["q"][ht][:], proj["k"][ht][:], True, zw_pool)
            turn_transposes(zc[:], ht, J_r, tp_psum)
        inverse_tail("r", J_r, rbufbig, (zw_pool, iv_psum, ct_pool))
    _pk_cm.__exit__(None, None, None)

    # ---------------- top-k + softmax + spike build ------------------------
    # S spectra slots (shared with the dead zcat tiles); the S time-domain
    # transposes are interleaved into the top-k loop so the PE has work
    # during the DVE-serial scans.
    spS = [
        spS_pool.tile([128, 16 * NC2], f16, name=f"sp_S_{rc}", tag="zsp", bufs=4)
        for rc in range(4)
    ]
    _st_cm = tc.tile_pool(name="strans", bufs=1)
    st_pool = _st_cm.__enter__()
    _stp_cm = tc.tile_pool(name="stpsum", bufs=1, space="PSUM")
    st_psum = _stp_cm.__enter__()
    # scratch cols ordered (j, hc, d) so the per-w load is a 3-dim DMA
    scratch = st_pool.tile([128, 4 * 16 * 128], f16, name="s_scr")
    svals = []
    with tc.tile_pool(name="topk", bufs=1) as tk_pool:
        for hc in range(4):
            sfx = hc % 2
            v8a = tk_pool.tile([128, 8], f16, name=f"v8a{hc}", tag=f"v8a{sfx}")
            v8b = tk_pool.tile([128, 8], f16, name=f"v8b{hc}", tag=f"v8b{sfx}")
            i8a = tk_pool.tile([128, 8], u32, name=f"i8a{hc}", tag=f"i8a{sfx}")
            i8b = tk_pool.tile([128, 8], u32, name=f"i8b{hc}", tag=f"i8b{sfx}")
            rscr = tk_pool.tile([128, L], f16, name=f"rscr{hc}", tag=f"rscr{sfx}")
            rview = rbufbig[:, hc * L : (hc + 1) * L]
            nc.vector.max(v8a[:], rview)
            nc.vector.max_index(i8a[:], v8a[:], rview)
            nc.vector.match_replace(rscr[:], v8a[:], rview, -60000.0)
            nc.vector.max(v8b[:], rscr[:])
            nc.vector.max_index(i8b[:], v8b[:], rscr[:])
            vals = tk_pool.tile([128, 16], f32, name=f"vals{hc}", tag=f"vals{sfx}")
            idxs = tk_pool.tile([128, 16], i32, name=f"idxs{hc}", tag=f"idxs{sfx}")
            nc.vector.tensor_copy(vals[:, 0:8], v8a[:])
            nc.vector.tensor_copy(vals[:, 8:16], v8b[:])
            nc.vector.tensor_copy(idxs[:, 0:8], i8a[:].bitcast(i32))
            nc.vector.tensor_copy(idxs[:, 8:16], i8b[:].bitcast(i32))
            # un-permute: c -> t = (c&15)*128 + (c>>7)*8 + ((c>>4)&7)
            tA = tk_pool.tile([128, 16], i32, name=f"tA{hc}", tag=f"tA{sfx}")
            tBv = tk_pool.tile([128, 16], i32, name=f"tB{hc}", tag=f"tB{sfx}")
            nc.vector.tensor_scalar(
                out=tA[:], in0=idxs[:], scalar1=15, scalar2=7,
                op0=ALU.bitwise_and, op1=ALU.logical_shift_left,
            )
            nc.vector.tensor_scalar(
                out=tBv[:], in0=idxs[:], scalar1=7, scalar2=3,
                op0=ALU.logical_shift_right, op1=ALU.logical_shift_left,
            )
            nc.vector.tensor_add(tA[:], tA[:], tBv[:])
            nc.vector.tensor_scalar(
                out=tBv[:], in0=idxs[:], scalar1=4, scalar2=7,
                op0=ALU.logical_shift_right, op1=ALU.bitwise_and,
            )
            nc.vector.tensor_add(tA[:], tA[:], tBv[:])
            # softmax over cols 0..14
            negmax = tk_pool.tile([128, 1], f32, name=f"negmax{hc}", tag=f"negmax{sfx}")
            nc.vector.tensor_scalar_mul(negmax[:], vals[:, 0:1], -1.0)
            e15 = tk_pool.tile([128, TOPK], f32, name=f"e15{hc}", tag=f"e15{sfx}")
            nc.scalar.activation(e15[:], vals[:, 0:TOPK], AF.Exp, bias=negmax[:], scale=1.0)
            s15 = tk_pool.tile([128, 1], f32, name=f"s15{hc}", tag=f"s15{sfx}")
            nc.vector.tensor_reduce(s15[:], e15[:], axis=AX.X, op=ALU.add)
            r15 = tk_pool.tile([128, 1], f32, name=f"r15{hc}", tag=f"r15{sfx}")
            nc.vector.reciprocal(r15[:], s15[:])
            wts = tk_pool.tile([128, 16], f16, name=f"wts{hc}", tag=f"wts{sfx}")
            nc.vector.memset(wts[:], 0.0)
            w15f = tk_pool.tile([128, TOPK], f32, name=f"w15f{hc}", tag=f"w15f{sfx}")
            nc.vector.tensor_scalar_mul(w15f[:], e15[:], r15[:])
            nc.vector.tensor_copy(wts[:, 0:TOPK], w15f[:])
            # pos = (2048 - t) & 2047 ; split halves with -1 padding
            pos = tk_pool.tile([128, 16], i32, name=f"pos{hc}", tag=f"pos{sfx}")
            nc.vector.tensor_scalar(out=pos[:], in0=tA[:], scalar1=-1, scalar2=2048, op0=ALU.mult, op1=ALU.add)
            nc.vector.tensor_scalar(out=pos[:], in0=pos[:], scalar1=2047, scalar2=None, op0=ALU.bitwise_and)
            mlt = tk_pool.tile([128, 16], i32, name=f"mlt{hc}", tag=f"mlt{sfx}")
            nc.vector.tensor_scalar(out=mlt[:], in0=pos[:], scalar1=1024, scalar2=None, op0=ALU.is_lt)
            mge = tk_pool.tile([128, 16], i32, name=f"mge{hc}", tag=f"mge{sfx}")
            nc.vector.tensor_scalar(out=mge[:], in0=pos[:], scalar1=1024, scalar2=None, op0=ALU.is_ge)
            # idx0 = pos*mlt + mlt - 1 ; idx1 = (pos-1024)*mge + mge - 1
            t0 = tk_pool.tile([128, 16], i32, name=f"t0{hc}", tag=f"t0{sfx}")
            nc.vector.tensor_mul(t0[:], pos[:], mlt[:])
            nc.vector.tensor_add(t0[:], t0[:], mlt[:])
            nc.vector.tensor_scalar_add(t0[:], t0[:], -1)
            t1b = tk_pool.tile([128, 16], i32, name=f"t1b{hc}", tag=f"t1b{sfx}")
            nc.vector.tensor_scalar_add(t1b[:], pos[:], -1024)
            nc.vector.tensor_mul(t1b[:], t1b[:], mge[:])
            nc.vector.tensor_add(t1b[:], t1b[:], mge[:])
            nc.vector.tensor_scalar_add(t1b[:], t1b[:], -1)
            ix0 = tk_pool.tile([128, 16], i16, name=f"ix0{hc}", tag=f"ix0{sfx}")
            ix1 = tk_pool.tile([128, 16], i16, name=f"ix1{hc}", tag=f"ix1{sfx}")
            nc.vector.tensor_copy(ix0[:], t0[:])
            nc.vector.tensor_copy(ix1[:], t1b[:])
            # build S rows for this hd chunk: [128, 2048] fp16
            s_sb = svrb_pool.tile([128, L], f16, name=f"s_sb{hc}")
            nc.gpsimd.local_scatter(s_sb[:, 0:1024], wts[:], ix0[:], 128, 1024, 16)
            nc.gpsimd.local_scatter(s_sb[:, 1024:2048], wts[:], ix1[:], 128, 1024, 16)
            svals.append(s_sb)
            for j in range(16):
                pt = st_psum.tile([128, 128], f16, name=f"spt{j % 2}", tag=f"spt{j % 2}")
                nc.tensor.transpose(pt[:], s_sb[:, j * 128 : (j + 1) * 128], ident[:])
                if j % 2 == 0:
                    nc.scalar.copy(scratch[:, (j * 4 + hc) * 128 : (j * 4 + hc) * 128 + 128], pt[:])
                else:
                    nc.vector.tensor_copy(scratch[:, (j * 4 + hc) * 128 : (j * 4 + hc) * 128 + 128], pt[:])
    _stp_cm.__exit__(None, None, None)

    # ---------------- S rfft -----------------------------------------------
    if True:
        with tc.tile_pool(name="xas", bufs=1) as xas_pool, \
             tc.tile_pool(name="sfwork", bufs=1) as sw_pool, \
             tc.tile_pool(name="sfpsum", bufs=1, space="PSUM") as sf_psum:
            xtilesS = {}

            def get_xviewS(w):
                for ww in (w, w + 1, w + 2, w + 3):
                    if ww in xtilesS or ww >= NW:
                        continue
                    xa = xas_pool.tile(
                        [128, D], f16, name=f"xaS{ww}", tag="xaS", bufs=4
                    )
                    nc.sync.dma_start(
                        out=xa[:],
                        in_=scratch[:].rearrange("p (j c) -> p j c", j=16)[
                            8 * ww : 8 * ww + 8
                        ],
                    )
                    xtilesS[ww] = xa
                return xtilesS[w][:]

            bbuf_S = big_pool.tile([128, 2 * NW * 512], f16, name="bbuf_S", tag="big")
            for s in stageA_steps("S", get_xviewS, m_re16, m_im16, sw_pool, sf_psum, bbuf_S):
                s()
            for s in stageB_steps("S", spS, bbuf_S, sf_psum):
                s()
    _st_cm.__exit__(None, None, None)
    _svrb_cm.__exit__(None, None, None)

    # ---------------- Y = Qhf * Sf, inverse, output proj -------------------
    _ag_cm = tc.tile_pool(name="aggp", bufs=1)
    ag_pool = _ag_cm.__enter__()
    aggbig = big_pool.tile([128, 4 * L], f16, name="aggbig", tag="big")
    with tc.tile_pool(name="ywork", bufs=1) as yw_pool, \
         tc.tile_pool(name="typsum", bufs=1, space="PSUM") as ty_psum, \
         tc.tile_pool(name="ivpsumy", bufs=1, space="PSUM") as iv_psumy, \
         tc.tile_pool(name="ctpooly", bufs=1) as ct_pooly, \
         tc.tile_pool(name="outp", bufs=2) as out_pool, \
         tc.tile_pool(name="outpsum", bufs=1, space="PSUM") as out_psum:
        outv = out_ext[:].rearrange(
            "(t1 wl t2l) d -> wl t2l t1 d", t1=16, wl=16, t2l=8
        )

        def out_unit(j):
            # out-proj column block j depends only on stage-2 chunk wp=j
            po = out_psum.tile([128, D], f32, name=f"po{j % 2}", tag=f"po{j % 2}")
            for hc in range(4):
                nc.tensor.matmul(
                    po[:],
                    aggbig[:, hc * L + j * 128 : hc * L + (j + 1) * 128],
                    woc[:, hc * D : (hc + 1) * D],
                    start=(hc == 0),
                    stop=(hc == 3),
                )
            ot = out_pool.tile([128, D], f32, name=f"ot{j % 2}", tag=f"ot{j % 2}")
            nc.vector.tensor_add(ot[:], po[:], bo_sb[:])
            nc.sync.dma_start(out=outv[j], in_=ot[:])

        J_y = yw_pool.tile([128, NW * 512], f16, name="J_y")
        for ht in range(4):
            yc = yw_pool.tile([128, 16 * 128], f16, name=f"ycat{ht}", tag=f"ycat{ht}")
            specmul(yc[:], proj["q"][ht][:], spS[ht][:], False, yw_pool)
            turn_transposes(yc[:], ht, J_y, ty_psum)
        inverse_tail("y", J_y, aggbig, (yw_pool, iv_psumy, ct_pooly))
        for j in range(NW):
            out_unit(j)

    _ag_cm.__exit__(None, None, None)
    _pk2 = None  # placeholder
    _spS_cm.__exit__(None, None, None)
    _phq_cm.__exit__(None, None, None)
    _big_cm.__exit__(None, None, None)


@functools.lru_cache(maxsize=1)
def _get_built():
    return _build()


def build_base_inputs(inputs):
    cs = _consts()
    return {
        "wq16": np.asarray(inputs["Wq"], np.float16),
        "wk16": np.asarray(inputs["Wk"], np.float16),
        "wo16": np.asarray(inputs["Wo"], np.float16),
        "bq2": np.asarray(inputs["bq"], np.float32).reshape(4, 128),
        "bk2": np.asarray(inputs["bk"], np.float32).reshape(4, 128),
        "borep": np.broadcast_to(
            np.asarray(inputs["bo"], np.float32), (128, D)
        ).copy(),
        "mre": cs["mre"], "mim": cs["mim"],
        "mre16": cs["mre16"], "mim16": cs["mim16"],
        "vcat": cs["vcat"], "smallc": cs["smallc"],
        "w216": cs["w216"],
    }


def kernel(**inputs):
    q = np.asarray(inputs["q"], np.float32)
    k = np.asarray(inputs["k"], np.float32)
    nc, par = _get_built()
    base = build_base_inputs(inputs)
    in_maps = [dict(base, q=q[b], k=k[b]) for b in range(NCORES)]
    res = run_bass_kernel_spmd(nc, in_maps, list(range(NCORES)))
    out = np.stack([res.results[b]["out"] for b in range(NCORES)], axis=0)
    return out.astype(np.float32)


if __name__ == "__main__":
    import reference  # noqa: F401

    ins = {k: np.asarray(v) for k, v in reference.setup_inputs().items()}
    got = kernel(**ins)
    exp = np.asarray(reference.reference(**ins))
    rel = np.linalg.norm(got - exp) / np.linalg.norm(exp)
    print("rel err:", rel)
